# revision 10
# baseline (speedup 1.0000x reference)
"""Trainium2 Bass kernel for nn_Decay (gated decay-memory block).

  gate  = sigmoid(x @ Wg + bg)
  store = (x @ Wv) * gate * scale          scale = sqrt(1 - decay)
  mem   = decay-scan(store)                y_t = store_t + decay * y_{t-1}
  que   = sigmoid(x @ Wq + bq)
  out   = (mem * que * scale) @ Wo

Sharding (8 cores): core c handles batch b = c//2, token half h = c%2
(2048 output tokens each).  The decay scan needs history: each core
computes a 512-token halo before its token range (zero-padded for h=0,
so all cores run the identical program).  decay^512 ~ 4e-12 => exact to
fp32 precision.  No collectives.

Layout: everything on-chip lives as [feature (partitions), token (free)].
 - projections:  out[m_tile, t_blk] = sum_ec Wx[ec, m_tile].T @ xT[ec, t_blk]
   (weights in natural [E, M] layout; x transposed host-side)
 - decay scan: DVE tensor_tensor_scan along the free (token) axis
 - output proj consumes load0 [m, t] directly; result is outT [E, T],
   un-transposed host-side.
Matmuls run in float32r (TF32-like, full PE rate at N>=256).

Phases (weights resident in SBUF, activations streamed):
  A0..A3: m-quarter q of {Wv*scale, Wg, Wq} resident; computes
          pv, gate, store, mem(scan), que, load0 -> spill load0 (f32r)
  C0..C3: e-quarter of Wo*scale resident; outT[e, t] = sum_m Wo.T @ load0
Weight tiles use separate tags; the first-needed one (wv / wo) has
bufs=2 so the next phase's load overlaps the current phase's compute.
"""

import sys, types

sys.path.insert(0, "/opt/trn_rl_repo")

import numpy as np

import concourse.bass as bass
import concourse.tile as tile
from concourse import bacc, mybir
from concourse.bass_utils import run_bass_kernel_spmd

# Problem constants (hardcoded per harness contract)
B, S, E, M = 4, 4096, 2048, 2048
DECAY = 0.95
SCALE = float(np.sqrt(1.0 - DECAY))

N_CORES = 8
HALO = 256            # halo tokens ahead of each core's range (decay^256 ~ 2e-6)
OUT_T = S // 2        # output tokens per core
T = OUT_T + HALO      # computed tokens per core
TB = 256              # token block (matmul free dim)
NTB = T // TB         # 10
OTB = OUT_T // TB     # 8 output-token blocks
OTB0 = HALO // TB     # first t-block holding output tokens
P = 128
EC = E // P           # 16 contraction chunks
MT = M // P           # 16 m tiles
MQ = 4                # m-quarters
MT_Q = MT // MQ       # 4 m-tiles per quarter
MQW = MT_Q * P        # 512
F32 = mybir.dt.float32
F32R = mybir.dt.float32r


def build_module(has_bias):
    nc = bacc.Bacc()

    xT_d = nc.dram_tensor("xT", [E, T], F32R, kind="ExternalInput")
    wv_d = nc.dram_tensor("Wvs", [E, M], F32R, kind="ExternalInput")
    wg_d = nc.dram_tensor("Wg", [E, M], F32R, kind="ExternalInput")
    wq_d = nc.dram_tensor("Wq", [E, M], F32R, kind="ExternalInput")
    wo_d = nc.dram_tensor("Wos", [M, E], F32R, kind="ExternalInput")
    if has_bias:
        bg_d = nc.dram_tensor("bg", [M], F32, kind="ExternalInput")
        bq_d = nc.dram_tensor("bq", [M], F32, kind="ExternalInput")
    outT_d = nc.dram_tensor("outT", [E, OUT_T], F32, kind="ExternalOutput")
    l0_buf = nc.dram_tensor("l0_buf", [M, OUT_T], F32R)  # internal spill

    with tile.TileContext(nc) as tc:
        with (
            tc.tile_pool(name="wva", bufs=2) as wvp,   # Wv quarters + Wo quarters
            tc.tile_pool(name="wgp", bufs=1) as wgp,
            tc.tile_pool(name="wqp", bufs=1) as wqp,
            tc.tile_pool(name="big", bufs=2) as bigp,
            tc.tile_pool(name="ws", bufs=2) as wsp,
            tc.tile_pool(name="mems", bufs=2) as memp,
            tc.tile_pool(name="consts", bufs=1) as cp,
            tc.tile_pool(name="ps", bufs=2, space="PSUM") as ps,
        ):
            # consts: decay broadcast [:, :TB]; bg at [:, TB:TB+MT]; bq after
            consts = cp.tile([P, TB + 2 * MT], F32, tag="consts")
            nc.vector.memset(consts[:, 0:TB], DECAY)
            if has_bias:
                nc.sync.dma_start(
                    out=consts[:, TB : TB + MT],
                    in_=bg_d.rearrange("(c p) -> p c", p=P),
                )
                nc.sync.dma_start(
                    out=consts[:, TB + MT : TB + 2 * MT],
                    in_=bq_d.rearrange("(c p) -> p c", p=P),
                )
            decay_t = consts[:, 0:TB]

            xT_r = xT_d.rearrange("(c p) t -> p c t", p=P)
            l0_r = l0_buf.rearrange("(c p) t -> p c t", p=P)
            outT_r = outT_d.rearrange("(c p) t -> p c t", p=P)

            EQ = E // MQ  # 512
            ETQ = EQ // P  # 4 e-tiles per quarter
            wv_tiles = {}

            def load_wv(q):
                # Wv quarter; prefetched one phase ahead (tag has bufs=2)
                msl = slice(q * MQW, (q + 1) * MQW)
                t = wvp.tile([P, EC, MQW], F32R, tag="wv", name=f"wv{q}")
                nc.scalar.dma_start(
                    out=t, in_=wv_d[:, msl].rearrange("(c p) m -> p c m", p=P)
                )
                wv_tiles[("v", q)] = t

            def load_wo(eq):
                # Wo e-quarter; shares the wv tag / prefetch slot
                esl = slice(eq * EQ, (eq + 1) * EQ)
                t = wvp.tile([P, MT, EQ], F32R, tag="wv", name=f"wo{eq}")
                nc.scalar.dma_start(
                    out=t, in_=wo_d[:, esl].rearrange("(c p) e -> p c e", p=P)
                )
                wv_tiles[("o", eq)] = t

            # ---- Phases A0..A3: quarter q of m ----
            for q in range(MQ):
                msl = slice(q * MQW, (q + 1) * MQW)
                if q == 0:
                    # startup: first x block, then weights in order of need
                    xt0 = bigp.tile([P, EC, TB], F32R, tag="t16", name="xt0_0")
                    nc.sync.dma_start(out=xt0, in_=xT_r[:, :, 0:TB])
                    load_wv(0)
                wv = wv_tiles[("v", q)]
                wg = wgp.tile([P, EC, MQW], F32R, tag="wg", name=f"wg{q}")
                wq = wqp.tile([P, EC, MQW], F32R, tag="wq", name=f"wq{q}")
                nc.sync.dma_start(
                    out=wg, in_=wg_d[:, msl].rearrange("(c p) m -> p c m", p=P)
                )
                nc.gpsimd.dma_start(
                    out=wq, in_=wq_d[:, msl].rearrange("(c p) m -> p c m", p=P)
                )
                mem_prev = None
                for tb in range(NTB):
                    if tb == 4:
                        # software prefetch of the next phase's first weight
                        if q + 1 < MQ:
                            load_wv(q + 1)
                        else:
                            load_wo(0)
                    tsl = slice(tb * TB, (tb + 1) * TB)
                    if q == 0 and tb == 0:
                        xt = xt0
                    else:
                        xt = bigp.tile([P, EC, TB], F32R, tag="t16", name=f"xt{q}_{tb}")
                        nc.sync.dma_start(out=xt, in_=xT_r[:, :, tsl])
                    need_q = tb >= OTB0
                    mem_t = memp.tile([P, MT_Q, TB], F32, tag="mem", name=f"mem{q}_{tb}")
                    if need_q:
                        l0_t = memp.tile(
                            [P, MT_Q, TB], F32R, tag="l0", name=f"l0{q}_{tb}"
                        )
                    pvs = []
                    for mt in range(MT_Q):
                        pv = ps.tile(
                            [P, TB], F32, tag="pv", bufs=5, name=f"pv{q}_{tb}_{mt}"
                        )
                        for ec in range(EC):
                            nc.tensor.matmul(
                                pv, lhsT=wv[:, ec, wsl := slice(mt * P, (mt + 1) * P)],
                                rhs=xt[:, ec, :],
                                start=(ec == 0), stop=(ec == EC - 1),
                            )
                        pvs.append(pv)
                    if need_q:
                        que_t = wsp.tile(
                            [P, MT_Q, TB], F32, tag="que", name=f"que{q}_{tb}"
                        )
                    for mt in range(MT_Q):
                        mtg = q * MT_Q + mt  # global m tile
                        wsl = slice(mt * P, (mt + 1) * P)
                        ws = wsp.tile([P, 2, TB], F32, tag="ws", name=f"ws{q}_{tb}_{mt}")
                        gate, store = ws[:, 0, :], ws[:, 1, :]
                        pg = ps.tile([P, TB], F32, tag="pg", name=f"pg{q}_{tb}_{mt}")
                        for ec in range(EC):
                            nc.tensor.matmul(
                                pg, lhsT=wg[:, ec, wsl], rhs=xt[:, ec, :],
                                start=(ec == 0), stop=(ec == EC - 1),
                            )
                        nc.scalar.activation(
                            gate, pg, mybir.ActivationFunctionType.Sigmoid,
                            bias=consts[:, TB + mtg : TB + mtg + 1] if has_bias else 0.0,
                        )
                        nc.vector.tensor_mul(store, pvs[mt], gate)
                        nc.vector.tensor_tensor_scan(
                            mem_t[:, mt, :], decay_t, store,
                            initial=0.0 if tb == 0 else mem_prev[:, mt, TB - 1 : TB],
                            op0=mybir.AluOpType.mult, op1=mybir.AluOpType.add,
                        )
                    if need_q:
                        for mt in range(MT_Q):
                            mtg = q * MT_Q + mt
                            wsl = slice(mt * P, (mt + 1) * P)
                            que = que_t[:, mt, :]
                            pq = ps.tile(
                                [P, TB], F32, tag="pv", bufs=5, name=f"pq{q}_{tb}_{mt}"
                            )
                            for ec in range(EC):
                                nc.tensor.matmul(
                                    pq, lhsT=wq[:, ec, wsl], rhs=xt[:, ec, :],
                                    start=(ec == 0), stop=(ec == EC - 1),
                                )
                            nc.scalar.activation(
                                que, pq, mybir.ActivationFunctionType.Sigmoid,
                                bias=consts[:, TB + MT + mtg : TB + MT + mtg + 1]
                                if has_bias else 0.0,
                            )
                            nc.vector.tensor_mul(l0_t[:, mt, :], mem_t[:, mt, :], que)
                        osl = slice((tb - OTB0) * TB, (tb - OTB0 + 1) * TB)
                        nc.gpsimd.dma_start(
                            out=l0_r[:, q * MT_Q : (q + 1) * MT_Q, osl], in_=l0_t
                        )
                    mem_prev = mem_t

            # ---- Phases C0..C3: e-quarter of the output projection ----
            for eq in range(MQ):
                wo = wv_tiles[("o", eq)]
                for tb in range(OTB):
                    if tb == 1 and eq + 1 < MQ:
                        load_wo(eq + 1)
                    tsl = slice(tb * TB, (tb + 1) * TB)
                    lt = bigp.tile([P, MT, TB], F32R, tag="t16", name=f"lt{eq}_{tb}")
                    nc.sync.dma_start(out=lt, in_=l0_r[:, :, tsl])
                    ot = memp.tile([P, ETQ, TB], F32, tag="mem", name=f"ot{eq}_{tb}")
                    for et in range(ETQ):
                        po = ps.tile([P, TB], F32, tag="pg", name=f"po{eq}_{tb}_{et}")
                        for mc in range(MT):
                            nc.tensor.matmul(
                                po, lhsT=wo[:, mc, et * P : (et + 1) * P],
                                rhs=lt[:, mc, :],
                                start=(mc == 0), stop=(mc == MT - 1),
                            )
                        nc.vector.tensor_copy(ot[:, et, :], po)
                    nc.gpsimd.dma_start(
                        out=outT_r[:, eq * ETQ : (eq + 1) * ETQ, tsl], in_=ot
                    )
    nc.compile()
    return nc


_cached = {}


def _get_module(has_bias):
    if has_bias not in _cached:
        _cached[has_bias] = build_module(has_bias)
    return _cached[has_bias]


def _prep_inputs(x, Wv, Wg, bg, Wq, bq, Wo, has_bias):
    """Shard + lay out host-side. Returns per-core input dicts."""
    Wvs = (Wv * SCALE).astype(np.float32)
    Wos = (Wo * SCALE).astype(np.float32)
    Wg = np.ascontiguousarray(Wg, dtype=np.float32)
    Wq = np.ascontiguousarray(Wq, dtype=np.float32)
    in_maps = []
    for c in range(N_CORES):
        b, h = c // 2, c % 2
        xTc = np.zeros((E, T), dtype=np.float32)
        start = h * OUT_T - HALO
        src = np.ascontiguousarray(x[b, max(start, 0) : h * OUT_T + OUT_T].T)
        xTc[:, T - src.shape[1] :] = src
        m = {"xT": xTc, "Wvs": Wvs, "Wg": Wg, "Wq": Wq, "Wos": Wos}
        if has_bias:
            m["bg"] = np.ascontiguousarray(bg, dtype=np.float32)
            m["bq"] = np.ascontiguousarray(bq, dtype=np.float32)
        in_maps.append(m)
    return in_maps


def run(x, Wv, Wg, bg, Wq, bq, Wo, trace=False):
    has_bias = bool(np.any(bg)) or bool(np.any(bq))
    nc = _get_module(has_bias)
    in_maps = _prep_inputs(x, Wv, Wg, bg, Wq, bq, Wo, has_bias)
    res = run_bass_kernel_spmd(
        nc, in_maps, core_ids=list(range(N_CORES)), trace=trace
    )
    out = np.empty((B, S, E), dtype=np.float32)
    for c in range(N_CORES):
        b, h = c // 2, c % 2
        out[b, h * OUT_T : (h + 1) * OUT_T] = res.results[c]["outT"].T
    return out, res


def kernel(**inputs):
    out, _ = run(**inputs)
    return out


# revision 13
# speedup vs baseline: 1.0787x; 1.0787x over previous
"""Trainium2 Bass kernel for nn_Decay (gated decay-memory block).

  gate  = sigmoid(x @ Wg + bg)
  store = (x @ Wv) * gate * scale          scale = sqrt(1 - decay)
  mem   = decay-scan(store)                y_t = store_t + decay * y_{t-1}
  que   = sigmoid(x @ Wq + bq)
  out   = (mem * que * scale) @ Wo

Sharding (8 cores): core c handles batch b = c//2, token half h = c%2
(2048 output tokens each).  The decay scan needs history: each core
computes a 512-token halo before its token range (zero-padded for h=0,
so all cores run the identical program).  decay^512 ~ 4e-12 => exact to
fp32 precision.  No collectives.

Layout: everything on-chip lives as [feature (partitions), token (free)].
 - projections:  out[m_tile, t_blk] = sum_ec Wx[ec, m_tile].T @ xT[ec, t_blk]
   (weights in natural [E, M] layout; x transposed host-side)
 - decay scan: DVE tensor_tensor_scan along the free (token) axis
 - output proj consumes load0 [m, t] directly; result is outT [E, T],
   un-transposed host-side.
Matmuls run in float32r (TF32-like, full PE rate at N>=256).

Phases (weights resident in SBUF, activations streamed):
  A0..A3: m-quarter q of {Wv*scale, Wg, Wq} resident; computes
          pv, gate, store, mem(scan), que, load0 -> spill load0 (f32r)
  C0..C3: e-quarter of Wo*scale resident; outT[e, t] = sum_m Wo.T @ load0
Weight tiles use separate tags; the first-needed one (wv / wo) has
bufs=2 so the next phase's load overlaps the current phase's compute.
"""

import sys, types

sys.path.insert(0, "/opt/trn_rl_repo")

import numpy as np

import concourse.bass as bass
import concourse.tile as tile
from concourse import bacc, mybir
from concourse.bass_utils import run_bass_kernel_spmd

# Problem constants (hardcoded per harness contract)
B, S, E, M = 4, 4096, 2048, 2048
DECAY = 0.95
SCALE = float(np.sqrt(1.0 - DECAY))

N_CORES = 8
HALO = 256            # halo tokens ahead of each core's range (decay^256 ~ 2e-6)
OUT_T = S // 2        # output tokens per core
T = OUT_T + HALO      # computed tokens per core
TB = 256              # token block (matmul free dim)
NTB = T // TB         # 10
OTB = OUT_T // TB     # 8 output-token blocks
OTB0 = HALO // TB     # first t-block holding output tokens
P = 128
EC = E // P           # 16 contraction chunks
MT = M // P           # 16 m tiles
MQ = 4                # m-quarters
MT_Q = MT // MQ       # 4 m-tiles per quarter
MQW = MT_Q * P        # 512
F32 = mybir.dt.float32
F32R = mybir.dt.float32r


def build_module(has_bias):
    nc = bacc.Bacc()

    xT_d = nc.dram_tensor("xT", [E, T], F32R, kind="ExternalInput")
    wv_d = nc.dram_tensor("Wvs", [E, M], F32R, kind="ExternalInput")
    wg_d = nc.dram_tensor("Wg", [E, M], F32R, kind="ExternalInput")
    wq_d = nc.dram_tensor("Wq", [E, M], F32R, kind="ExternalInput")
    wo_d = nc.dram_tensor("Wos", [M, E], F32R, kind="ExternalInput")
    if has_bias:
        bg_d = nc.dram_tensor("bg", [M], F32, kind="ExternalInput")
        bq_d = nc.dram_tensor("bq", [M], F32, kind="ExternalInput")
    outT_d = nc.dram_tensor("outT", [E, OUT_T], F32, kind="ExternalOutput")
    l0_buf = nc.dram_tensor("l0_buf", [M, OUT_T], F32R)  # internal spill

    with tile.TileContext(nc) as tc:
        with (
            tc.tile_pool(name="wva", bufs=2) as wvp,   # Wv quarters + Wo quarters
            tc.tile_pool(name="wgp", bufs=1) as wgp,
            tc.tile_pool(name="wqp", bufs=1) as wqp,
            tc.tile_pool(name="big", bufs=2) as bigp,
            tc.tile_pool(name="ws", bufs=2) as wsp,
            tc.tile_pool(name="mems", bufs=2) as memp,
            tc.tile_pool(name="consts", bufs=1) as cp,
            tc.tile_pool(name="ps", bufs=2, space="PSUM") as ps,
        ):
            # consts: decay broadcast [:, :TB]; bg at [:, TB:TB+MT]; bq after
            consts = cp.tile([P, TB + 2 * MT], F32, tag="consts")
            nc.vector.memset(consts[:, 0:TB], DECAY)
            if has_bias:
                nc.sync.dma_start(
                    out=consts[:, TB : TB + MT],
                    in_=bg_d.rearrange("(c p) -> p c", p=P),
                )
                nc.sync.dma_start(
                    out=consts[:, TB + MT : TB + 2 * MT],
                    in_=bq_d.rearrange("(c p) -> p c", p=P),
                )
            decay_t = consts[:, 0:TB]

            xT_r = xT_d.rearrange("(c p) t -> p c t", p=P)
            l0_r = l0_buf.rearrange("(c p) t -> p c t", p=P)
            outT_r = outT_d.rearrange("(c p) t -> p c t", p=P)

            EQ = E // MQ  # 512
            ETQ = EQ // P  # 4 e-tiles per quarter
            wv_tiles = {}

            def load_wv(q):
                # Wv quarter; prefetched one phase ahead (tag has bufs=2)
                msl = slice(q * MQW, (q + 1) * MQW)
                t = wvp.tile([P, EC, MQW], F32R, tag="wv", name=f"wv{q}")
                nc.scalar.dma_start(
                    out=t, in_=wv_d[:, msl].rearrange("(c p) m -> p c m", p=P)
                )
                wv_tiles[("v", q)] = t

            def load_wo(eq):
                # Wo e-quarter; shares the wv tag / prefetch slot
                esl = slice(eq * EQ, (eq + 1) * EQ)
                t = wvp.tile([P, MT, EQ], F32R, tag="wv", name=f"wo{eq}")
                nc.scalar.dma_start(
                    out=t, in_=wo_d[:, esl].rearrange("(c p) e -> p c e", p=P)
                )
                wv_tiles[("o", eq)] = t

            # ---- Phases A0..A3: quarter q of m ----
            for q in range(MQ):
                msl = slice(q * MQW, (q + 1) * MQW)
                if q == 0:
                    # startup: first x block, then weights in order of need
                    xt0 = bigp.tile([P, EC, TB], F32R, tag="t16", name="xt0_0")
                    nc.sync.dma_start(out=xt0, in_=xT_r[:, :, 0:TB])
                    load_wv(0)
                wv = wv_tiles[("v", q)]
                wg = wgp.tile([P, EC, MQW], F32R, tag="wg", name=f"wg{q}")
                wq = wqp.tile([P, EC, MQW], F32R, tag="wq", name=f"wq{q}")
                nc.scalar.dma_start(
                    out=wg, in_=wg_d[:, msl].rearrange("(c p) m -> p c m", p=P)
                )
                nc.gpsimd.dma_start(
                    out=wq, in_=wq_d[:, msl].rearrange("(c p) m -> p c m", p=P)
                )
                mem_prev = None
                for tb in range(NTB):
                    if tb == 4:
                        # software prefetch of the next phase's first weight
                        if q + 1 < MQ:
                            load_wv(q + 1)
                        else:
                            load_wo(0)
                    tsl = slice(tb * TB, (tb + 1) * TB)
                    if q == 0 and tb == 0:
                        xt = xt0
                    else:
                        xt = bigp.tile([P, EC, TB], F32R, tag="t16", name=f"xt{q}_{tb}")
                        nc.sync.dma_start(out=xt, in_=xT_r[:, :, tsl])
                    need_q = tb >= OTB0
                    mem_t = memp.tile([P, MT_Q, TB], F32, tag="mem", name=f"mem{q}_{tb}")
                    if need_q:
                        l0_t = memp.tile(
                            [P, MT_Q, TB], F32R, tag="l0", name=f"l0{q}_{tb}"
                        )
                    pvs = []
                    for mt in range(MT_Q):
                        pv = ps.tile(
                            [P, TB], F32, tag="pv", bufs=5, name=f"pv{q}_{tb}_{mt}"
                        )
                        for ec in range(EC):
                            nc.tensor.matmul(
                                pv, lhsT=wv[:, ec, wsl := slice(mt * P, (mt + 1) * P)],
                                rhs=xt[:, ec, :],
                                start=(ec == 0), stop=(ec == EC - 1),
                            )
                        pvs.append(pv)
                    if need_q:
                        que_t = wsp.tile(
                            [P, MT_Q, TB], F32, tag="que", name=f"que{q}_{tb}"
                        )
                    for mt in range(MT_Q):
                        mtg = q * MT_Q + mt  # global m tile
                        wsl = slice(mt * P, (mt + 1) * P)
                        ws = wsp.tile([P, 2, TB], F32, tag="ws", name=f"ws{q}_{tb}_{mt}")
                        gate, store = ws[:, 0, :], ws[:, 1, :]
                        pg = ps.tile([P, TB], F32, tag="pg", name=f"pg{q}_{tb}_{mt}")
                        for ec in range(EC):
                            nc.tensor.matmul(
                                pg, lhsT=wg[:, ec, wsl], rhs=xt[:, ec, :],
                                start=(ec == 0), stop=(ec == EC - 1),
                            )
                        nc.scalar.activation(
                            gate, pg, mybir.ActivationFunctionType.Sigmoid,
                            bias=consts[:, TB + mtg : TB + mtg + 1] if has_bias else 0.0,
                        )
                        nc.vector.tensor_mul(store, pvs[mt], gate)
                        nc.vector.tensor_tensor_scan(
                            mem_t[:, mt, :], decay_t, store,
                            initial=0.0 if tb == 0 else mem_prev[:, mt, TB - 1 : TB],
                            op0=mybir.AluOpType.mult, op1=mybir.AluOpType.add,
                        )
                    if need_q:
                        for mt in range(MT_Q):
                            mtg = q * MT_Q + mt
                            wsl = slice(mt * P, (mt + 1) * P)
                            que = que_t[:, mt, :]
                            pq = ps.tile(
                                [P, TB], F32, tag="pv", bufs=5, name=f"pq{q}_{tb}_{mt}"
                            )
                            for ec in range(EC):
                                nc.tensor.matmul(
                                    pq, lhsT=wq[:, ec, wsl], rhs=xt[:, ec, :],
                                    start=(ec == 0), stop=(ec == EC - 1),
                                )
                            nc.scalar.activation(
                                que, pq, mybir.ActivationFunctionType.Sigmoid,
                                bias=consts[:, TB + MT + mtg : TB + MT + mtg + 1]
                                if has_bias else 0.0,
                            )
                            nc.vector.tensor_mul(l0_t[:, mt, :], mem_t[:, mt, :], que)
                        osl = slice((tb - OTB0) * TB, (tb - OTB0 + 1) * TB)
                        nc.gpsimd.dma_start(
                            out=l0_r[:, q * MT_Q : (q + 1) * MT_Q, osl], in_=l0_t
                        )
                    mem_prev = mem_t

            # ---- Phases C: output projection, e-quarter PAIRS resident ----
            # token-block outer so each l0 block is read once per pair
            for cp in range(2):
                eqs = (2 * cp, 2 * cp + 1)
                # wo(2cp) prefetched earlier; its pair partner loads here
                load_wo(2 * cp + 1)
                wos = [wv_tiles[("o", eq)] for eq in eqs]
                for tb in range(OTB):
                    if cp == 0 and tb == 1:
                        load_wo(2)  # waits for a free slot (end of cp0)
                    tsl = slice(tb * TB, (tb + 1) * TB)
                    lt = bigp.tile([P, MT, TB], F32R, tag="t16", name=f"lt{cp}_{tb}")
                    nc.sync.dma_start(out=lt, in_=l0_r[:, :, tsl])
                    for j, eq in enumerate(eqs):
                        ot = memp.tile(
                            [P, ETQ, TB], F32, tag=("mem", "l0")[j],
                            name=f"ot{eq}_{tb}",
                        )
                        for et in range(ETQ):
                            po = ps.tile([P, TB], F32, tag="pg", name=f"po{eq}_{tb}_{et}")
                            for mc in range(MT):
                                nc.tensor.matmul(
                                    po, lhsT=wos[j][:, mc, et * P : (et + 1) * P],
                                    rhs=lt[:, mc, :],
                                    start=(mc == 0), stop=(mc == MT - 1),
                                )
                            nc.vector.tensor_copy(ot[:, et, :], po)
                        nc.gpsimd.dma_start(
                            out=outT_r[:, eq * ETQ : (eq + 1) * ETQ, tsl], in_=ot
                        )
    nc.compile()
    return nc


_cached = {}


def _get_module(has_bias):
    if has_bias not in _cached:
        _cached[has_bias] = build_module(has_bias)
    return _cached[has_bias]


def _prep_inputs(x, Wv, Wg, bg, Wq, bq, Wo, has_bias):
    """Shard + lay out host-side. Returns per-core input dicts."""
    Wvs = (Wv * SCALE).astype(np.float32)
    Wos = (Wo * SCALE).astype(np.float32)
    Wg = np.ascontiguousarray(Wg, dtype=np.float32)
    Wq = np.ascontiguousarray(Wq, dtype=np.float32)
    in_maps = []
    for c in range(N_CORES):
        b, h = c // 2, c % 2
        xTc = np.zeros((E, T), dtype=np.float32)
        start = h * OUT_T - HALO
        src = np.ascontiguousarray(x[b, max(start, 0) : h * OUT_T + OUT_T].T)
        xTc[:, T - src.shape[1] :] = src
        m = {"xT": xTc, "Wvs": Wvs, "Wg": Wg, "Wq": Wq, "Wos": Wos}
        if has_bias:
            m["bg"] = np.ascontiguousarray(bg, dtype=np.float32)
            m["bq"] = np.ascontiguousarray(bq, dtype=np.float32)
        in_maps.append(m)
    return in_maps


def run(x, Wv, Wg, bg, Wq, bq, Wo, trace=False):
    has_bias = bool(np.any(bg)) or bool(np.any(bq))
    nc = _get_module(has_bias)
    in_maps = _prep_inputs(x, Wv, Wg, bg, Wq, bq, Wo, has_bias)
    res = run_bass_kernel_spmd(
        nc, in_maps, core_ids=list(range(N_CORES)), trace=trace
    )
    out = np.empty((B, S, E), dtype=np.float32)
    for c in range(N_CORES):
        b, h = c // 2, c % 2
        out[b, h * OUT_T : (h + 1) * OUT_T] = res.results[c]["outT"].T
    return out, res


def kernel(**inputs):
    out, _ = run(**inputs)
    return out


# revision 16
# speedup vs baseline: 1.0866x; 1.0074x over previous
"""Trainium2 Bass kernel for nn_Decay (gated decay-memory block).

  gate  = sigmoid(x @ Wg + bg)
  store = (x @ Wv) * gate * scale          scale = sqrt(1 - decay)
  mem   = decay-scan(store)                y_t = store_t + decay * y_{t-1}
  que   = sigmoid(x @ Wq + bq)
  out   = (mem * que * scale) @ Wo

Sharding (8 cores): core c handles batch b = c//2, token half h = c%2
(2048 output tokens each).  The decay scan needs history: each core
computes a 512-token halo before its token range (zero-padded for h=0,
so all cores run the identical program).  decay^512 ~ 4e-12 => exact to
fp32 precision.  No collectives.

Layout: everything on-chip lives as [feature (partitions), token (free)].
 - projections:  out[m_tile, t_blk] = sum_ec Wx[ec, m_tile].T @ xT[ec, t_blk]
   (weights in natural [E, M] layout; x transposed host-side)
 - decay scan: DVE tensor_tensor_scan along the free (token) axis
 - output proj consumes load0 [m, t] directly; result is outT [E, T],
   un-transposed host-side.
Matmuls run in float32r (TF32-like, full PE rate at N>=256).

Phases (weights resident in SBUF, activations streamed):
  A0..A3: m-quarter q of {Wv*scale, Wg, Wq} resident; computes
          pv, gate, store, mem(scan), que, load0 -> spill load0 (f32r)
  C0..C3: e-quarter of Wo*scale resident; outT[e, t] = sum_m Wo.T @ load0
Weight tiles use separate tags; the first-needed one (wv / wo) has
bufs=2 so the next phase's load overlaps the current phase's compute.
"""

import sys, types

sys.path.insert(0, "/opt/trn_rl_repo")

import numpy as np

import concourse.bass as bass
import concourse.tile as tile
from concourse import bacc, mybir
from concourse.bass_utils import run_bass_kernel_spmd

# Problem constants (hardcoded per harness contract)
B, S, E, M = 4, 4096, 2048, 2048
DECAY = 0.95
SCALE = float(np.sqrt(1.0 - DECAY))

N_CORES = 8
HALO = 256            # halo tokens ahead of each core's range (decay^256 ~ 2e-6)
OUT_T = S // 2        # output tokens per core
T = OUT_T + HALO      # computed tokens per core
TB = 256              # token block (matmul free dim)
NTB = T // TB         # 10
OTB = OUT_T // TB     # 8 output-token blocks
OTB0 = HALO // TB     # first t-block holding output tokens
P = 128
EC = E // P           # 16 contraction chunks
MT = M // P           # 16 m tiles
MQ = 4                # m-quarters
MT_Q = MT // MQ       # 4 m-tiles per quarter
MQW = MT_Q * P        # 512
F32 = mybir.dt.float32
F32R = mybir.dt.float32r


def build_module(has_bias):
    nc = bacc.Bacc()

    xT_d = nc.dram_tensor("xT", [E, T], F32R, kind="ExternalInput")
    wv_d = nc.dram_tensor("Wvs", [E, M], F32R, kind="ExternalInput")
    wg_d = nc.dram_tensor("Wg", [E, M], F32R, kind="ExternalInput")
    wq_d = nc.dram_tensor("Wq", [E, M], F32R, kind="ExternalInput")
    wo_d = nc.dram_tensor("Wos", [M, E], F32R, kind="ExternalInput")
    if has_bias:
        bg_d = nc.dram_tensor("bg", [M], F32, kind="ExternalInput")
        bq_d = nc.dram_tensor("bq", [M], F32, kind="ExternalInput")
    outT_d = nc.dram_tensor("outT", [E, OUT_T], F32, kind="ExternalOutput")
    l0_buf = nc.dram_tensor("l0_buf", [M, OUT_T], F32R)  # internal spill

    with tile.TileContext(nc) as tc:
        with (
            tc.tile_pool(name="wva", bufs=2) as wvp,   # Wv quarters + Wo quarters
            tc.tile_pool(name="wgap", bufs=2) as wgap,  # Wg half-quarters (prefetch)
            tc.tile_pool(name="wgbp", bufs=1) as wgbp,
            tc.tile_pool(name="wqp", bufs=1) as wqp,
            tc.tile_pool(name="big", bufs=2) as bigp,
            tc.tile_pool(name="ws", bufs=4) as wsp,
            tc.tile_pool(name="mems", bufs=2) as memp,
            tc.tile_pool(name="consts", bufs=1) as cp,
            tc.tile_pool(name="ps", bufs=2, space="PSUM") as ps,
        ):
            # consts: decay broadcast [:, :TB]; bg at [:, TB:TB+MT]; bq after
            consts = cp.tile([P, TB + 2 * MT], F32, tag="consts")
            nc.vector.memset(consts[:, 0:TB], DECAY)
            if has_bias:
                nc.sync.dma_start(
                    out=consts[:, TB : TB + MT],
                    in_=bg_d.rearrange("(c p) -> p c", p=P),
                )
                nc.sync.dma_start(
                    out=consts[:, TB + MT : TB + 2 * MT],
                    in_=bq_d.rearrange("(c p) -> p c", p=P),
                )
            decay_t = consts[:, 0:TB]

            xT_r = xT_d.rearrange("(c p) t -> p c t", p=P)
            l0_r = l0_buf.rearrange("(c p) t -> p c t", p=P)
            outT_r = outT_d.rearrange("(c p) t -> p c t", p=P)

            EQ = E // MQ  # 512
            ETQ = EQ // P  # 4 e-tiles per quarter
            wv_tiles = {}

            def load_wv(q):
                # Wv quarter; prefetched one phase ahead (tag has bufs=2)
                msl = slice(q * MQW, (q + 1) * MQW)
                t = wvp.tile([P, EC, MQW], F32R, tag="wv", name=f"wv{q}")
                nc.scalar.dma_start(
                    out=t, in_=wv_d[:, msl].rearrange("(c p) m -> p c m", p=P)
                )
                wv_tiles[("v", q)] = t

            def load_wo(eq):
                # Wo e-quarter; shares the wv tag / prefetch slot
                esl = slice(eq * EQ, (eq + 1) * EQ)
                t = wvp.tile([P, MT, EQ], F32R, tag="wv", name=f"wo{eq}")
                nc.scalar.dma_start(
                    out=t, in_=wo_d[:, esl].rearrange("(c p) e -> p c e", p=P)
                )
                wv_tiles[("o", eq)] = t

            def load_wga(q):
                # first half (m-tiles 0-1) of the Wg quarter; prefetched
                msl = slice(q * MQW, q * MQW + 2 * P)
                t = wgap.tile([P, EC, 2 * P], F32R, tag="wga", name=f"wga{q}")
                nc.scalar.dma_start(
                    out=t, in_=wg_d[:, msl].rearrange("(c p) m -> p c m", p=P)
                )
                wv_tiles[("gA", q)] = t

            # ---- Phases A0..A3: quarter q of m ----
            for q in range(MQ):
                msl = slice(q * MQW, (q + 1) * MQW)
                if q == 0:
                    # startup: first x block, then weights in order of need
                    xt0 = bigp.tile([P, EC, TB], F32R, tag="t16", name="xt0_0")
                    nc.sync.dma_start(out=xt0, in_=xT_r[:, :, 0:TB])
                    load_wv(0)
                    load_wga(0)
                wv = wv_tiles[("v", q)]
                wga = wv_tiles[("gA", q)]
                mem_prev = None
                for tb in range(NTB):
                    tsl = slice(tb * TB, (tb + 1) * TB)
                    if q == 0 and tb == 0:
                        xt = xt0
                    else:
                        xt = bigp.tile([P, EC, TB], F32R, tag="t16", name=f"xt{q}_{tb}")
                        nc.sync.dma_start(out=xt, in_=xT_r[:, :, tsl])
                    if tb == 0:
                        # rest of this phase's weights (after xt in queue order)
                        mslB = slice(q * MQW + 2 * P, (q + 1) * MQW)
                        wgb = wgbp.tile([P, EC, 2 * P], F32R, tag="wgb", name=f"wgb{q}")
                        nc.sync.dma_start(
                            out=wgb,
                            in_=wg_d[:, mslB].rearrange("(c p) m -> p c m", p=P),
                        )
                        wq = wqp.tile([P, EC, MQW], F32R, tag="wq", name=f"wq{q}")
                        nc.gpsimd.dma_start(
                            out=wq, in_=wq_d[:, msl].rearrange("(c p) m -> p c m", p=P)
                        )
                    if tb == 4:
                        # software prefetch of the next phase's first weights
                        if q + 1 < MQ:
                            load_wv(q + 1)
                            load_wga(q + 1)
                        else:
                            load_wo(0)
                    need_q = tb >= OTB0
                    mem_t = memp.tile([P, MT_Q, TB], F32, tag="mem", name=f"mem{q}_{tb}")
                    pvs = []
                    for mt in range(MT_Q):
                        pv = ps.tile(
                            [P, TB], F32, tag="pv", bufs=5, name=f"pv{q}_{tb}_{mt}"
                        )
                        for ec in range(EC):
                            nc.tensor.matmul(
                                pv, lhsT=wv[:, ec, wsl := slice(mt * P, (mt + 1) * P)],
                                rhs=xt[:, ec, :],
                                start=(ec == 0), stop=(ec == EC - 1),
                            )
                        pvs.append(pv)
                    wss = []
                    for mt in range(MT_Q):
                        mtg = q * MT_Q + mt  # global m tile
                        wsl = slice((mt % 2) * P, (mt % 2 + 1) * P)
                        wgt = wga if mt < 2 else wgb
                        ws = wsp.tile([P, 3, TB], F32R, tag="ws", name=f"ws{q}_{tb}_{mt}")
                        wss.append(ws)
                        gate, store = ws[:, 0, :], ws[:, 1, :]
                        pg = ps.tile([P, TB], F32, tag="pg", name=f"pg{q}_{tb}_{mt}")
                        for ec in range(EC):
                            nc.tensor.matmul(
                                pg, lhsT=wgt[:, ec, wsl], rhs=xt[:, ec, :],
                                start=(ec == 0), stop=(ec == EC - 1),
                            )
                        nc.scalar.activation(
                            gate, pg, mybir.ActivationFunctionType.Sigmoid,
                            bias=consts[:, TB + mtg : TB + mtg + 1] if has_bias else 0.0,
                        )
                        nc.vector.tensor_mul(store, pvs[mt], gate)
                        nc.vector.tensor_tensor_scan(
                            mem_t[:, mt, :], decay_t, store,
                            initial=0.0 if tb == 0 else mem_prev[:, mt, TB - 1 : TB],
                            op0=mybir.AluOpType.mult, op1=mybir.AluOpType.add,
                        )
                    if need_q:
                        osl = slice((tb - OTB0) * TB, (tb - OTB0 + 1) * TB)
                        for mt in range(MT_Q):
                            mtg = q * MT_Q + mt
                            wsl = slice(mt * P, (mt + 1) * P)
                            ws = wss[mt]
                            que, l0 = ws[:, 2, :], ws[:, 0, :]  # l0 reuses gate slot
                            pq = ps.tile(
                                [P, TB], F32, tag="pv", bufs=5, name=f"pq{q}_{tb}_{mt}"
                            )
                            for ec in range(EC):
                                nc.tensor.matmul(
                                    pq, lhsT=wq[:, ec, wsl], rhs=xt[:, ec, :],
                                    start=(ec == 0), stop=(ec == EC - 1),
                                )
                            nc.scalar.activation(
                                que, pq, mybir.ActivationFunctionType.Sigmoid,
                                bias=consts[:, TB + MT + mtg : TB + MT + mtg + 1]
                                if has_bias else 0.0,
                            )
                            nc.vector.tensor_mul(l0, mem_t[:, mt, :], que)
                            nc.gpsimd.dma_start(
                                out=l0_r[:, mtg : mtg + 1, osl],
                                in_=l0.unsqueeze(1),
                            )
                    mem_prev = mem_t

            # ---- Phases C: output projection, e-quarter PAIRS resident ----
            # token-block outer so each l0 block is read once per pair
            for cp in range(2):
                eqs = (2 * cp, 2 * cp + 1)
                # wo(2cp) prefetched earlier; its pair partner loads here
                load_wo(2 * cp + 1)
                wos = [wv_tiles[("o", eq)] for eq in eqs]
                for tb in range(OTB):
                    if cp == 0 and tb == 1:
                        load_wo(2)  # waits for a free slot (end of cp0)
                    tsl = slice(tb * TB, (tb + 1) * TB)
                    lt = bigp.tile([P, MT, TB], F32R, tag="t16", name=f"lt{cp}_{tb}")
                    nc.sync.dma_start(out=lt, in_=l0_r[:, :, tsl])
                    for j, eq in enumerate(eqs):
                        ot = memp.tile(
                            [P, ETQ, TB], F32, tag=("mem", "l0")[j],
                            name=f"ot{eq}_{tb}",
                        )
                        for et in range(ETQ):
                            po = ps.tile([P, TB], F32, tag="pg", name=f"po{eq}_{tb}_{et}")
                            for mc in range(MT):
                                nc.tensor.matmul(
                                    po, lhsT=wos[j][:, mc, et * P : (et + 1) * P],
                                    rhs=lt[:, mc, :],
                                    start=(mc == 0), stop=(mc == MT - 1),
                                )
                            nc.vector.tensor_copy(ot[:, et, :], po)
                        nc.gpsimd.dma_start(
                            out=outT_r[:, eq * ETQ : (eq + 1) * ETQ, tsl], in_=ot
                        )
    nc.compile()
    return nc


_cached = {}


def _get_module(has_bias):
    if has_bias not in _cached:
        _cached[has_bias] = build_module(has_bias)
    return _cached[has_bias]


def _prep_inputs(x, Wv, Wg, bg, Wq, bq, Wo, has_bias):
    """Shard + lay out host-side. Returns per-core input dicts."""
    Wvs = (Wv * SCALE).astype(np.float32)
    Wos = (Wo * SCALE).astype(np.float32)
    Wg = np.ascontiguousarray(Wg, dtype=np.float32)
    Wq = np.ascontiguousarray(Wq, dtype=np.float32)
    in_maps = []
    for c in range(N_CORES):
        b, h = c // 2, c % 2
        xTc = np.zeros((E, T), dtype=np.float32)
        start = h * OUT_T - HALO
        src = np.ascontiguousarray(x[b, max(start, 0) : h * OUT_T + OUT_T].T)
        xTc[:, T - src.shape[1] :] = src
        m = {"xT": xTc, "Wvs": Wvs, "Wg": Wg, "Wq": Wq, "Wos": Wos}
        if has_bias:
            m["bg"] = np.ascontiguousarray(bg, dtype=np.float32)
            m["bq"] = np.ascontiguousarray(bq, dtype=np.float32)
        in_maps.append(m)
    return in_maps


def run(x, Wv, Wg, bg, Wq, bq, Wo, trace=False):
    has_bias = bool(np.any(bg)) or bool(np.any(bq))
    nc = _get_module(has_bias)
    in_maps = _prep_inputs(x, Wv, Wg, bg, Wq, bq, Wo, has_bias)
    res = run_bass_kernel_spmd(
        nc, in_maps, core_ids=list(range(N_CORES)), trace=trace
    )
    out = np.empty((B, S, E), dtype=np.float32)
    for c in range(N_CORES):
        b, h = c // 2, c % 2
        out[b, h * OUT_T : (h + 1) * OUT_T] = res.results[c]["outT"].T
    return out, res


def kernel(**inputs):
    out, _ = run(**inputs)
    return out


# revision 18
# speedup vs baseline: 1.1035x; 1.0155x over previous
"""Trainium2 Bass kernel for nn_Decay (gated decay-memory block).

  gate  = sigmoid(x @ Wg + bg)
  store = (x @ Wv) * gate * scale          scale = sqrt(1 - decay)
  mem   = decay-scan(store)                y_t = store_t + decay * y_{t-1}
  que   = sigmoid(x @ Wq + bq)
  out   = (mem * que * scale) @ Wo

Sharding (8 cores): core c handles batch b = c//2, token half h = c%2
(2048 output tokens each).  The decay scan needs history: each core
computes a 512-token halo before its token range (zero-padded for h=0,
so all cores run the identical program).  decay^512 ~ 4e-12 => exact to
fp32 precision.  No collectives.

Layout: everything on-chip lives as [feature (partitions), token (free)].
 - projections:  out[m_tile, t_blk] = sum_ec Wx[ec, m_tile].T @ xT[ec, t_blk]
   (weights in natural [E, M] layout; x transposed host-side)
 - decay scan: DVE tensor_tensor_scan along the free (token) axis
 - output proj consumes load0 [m, t] directly; result is outT [E, T],
   un-transposed host-side.
Matmuls run in float32r (TF32-like, full PE rate at N>=256).

Phases (weights resident in SBUF, activations streamed):
  A0..A3: m-quarter q of {Wv*scale, Wg, Wq} resident; computes
          pv, gate, store, mem(scan), que, load0 -> spill load0 (f32r)
  C0..C3: e-quarter of Wo*scale resident; outT[e, t] = sum_m Wo.T @ load0
Weight tiles use separate tags; the first-needed one (wv / wo) has
bufs=2 so the next phase's load overlaps the current phase's compute.
"""

import sys, types

sys.path.insert(0, "/opt/trn_rl_repo")

import numpy as np

import concourse.bass as bass
import concourse.tile as tile
from concourse import bacc, mybir
from concourse.bass_utils import run_bass_kernel_spmd

# Problem constants (hardcoded per harness contract)
B, S, E, M = 4, 4096, 2048, 2048
DECAY = 0.95
SCALE = float(np.sqrt(1.0 - DECAY))

N_CORES = 8
HALO = 256            # halo tokens ahead of each core's range (decay^256 ~ 2e-6)
OUT_T = S // 2        # output tokens per core
T = OUT_T + HALO      # computed tokens per core
TB = 256              # token block (matmul free dim)
NTB = T // TB         # 10
OTB = OUT_T // TB     # 8 output-token blocks
OTB0 = HALO // TB     # first t-block holding output tokens
P = 128
EC = E // P           # 16 contraction chunks
MT = M // P           # 16 m tiles
MQ = 4                # m-quarters
MT_Q = MT // MQ       # 4 m-tiles per quarter
MQW = MT_Q * P        # 512
F32 = mybir.dt.float32
F32R = mybir.dt.float32r


def build_module(has_bias):
    nc = bacc.Bacc()

    xT_d = nc.dram_tensor("xT", [E, T], F32R, kind="ExternalInput")
    wv_d = nc.dram_tensor("Wvs", [E, M], F32R, kind="ExternalInput")
    wg_d = nc.dram_tensor("Wg", [E, M], F32R, kind="ExternalInput")
    wq_d = nc.dram_tensor("Wq", [E, M], F32R, kind="ExternalInput")
    wo_d = nc.dram_tensor("Wos", [M, E], F32R, kind="ExternalInput")
    if has_bias:
        bg_d = nc.dram_tensor("bg", [M], F32, kind="ExternalInput")
        bq_d = nc.dram_tensor("bq", [M], F32, kind="ExternalInput")
    outT_d = nc.dram_tensor("outT", [E, OUT_T], F32, kind="ExternalOutput")
    l0_buf = nc.dram_tensor("l0_buf", [M, OUT_T], F32R)  # internal spill

    with tile.TileContext(nc) as tc:
        with (
            tc.tile_pool(name="wva", bufs=2) as wvp,   # Wv quarters + Wo quarters
            tc.tile_pool(name="wgap", bufs=2) as wgap,  # Wg half-quarters (prefetch)
            tc.tile_pool(name="wgbp", bufs=1) as wgbp,
            tc.tile_pool(name="wqp", bufs=1) as wqp,
            tc.tile_pool(name="big", bufs=2) as bigp,
            tc.tile_pool(name="ws", bufs=4) as wsp,
            tc.tile_pool(name="mems", bufs=2) as memp,
            tc.tile_pool(name="consts", bufs=1) as cp,
            tc.tile_pool(name="ps", bufs=2, space="PSUM") as ps,
        ):
            # consts: decay broadcast [:, :TB]; bg at [:, TB:TB+MT]; bq after
            consts = cp.tile([P, TB + 2 * MT], F32, tag="consts")
            nc.vector.memset(consts[:, 0:TB], DECAY)
            if has_bias:
                nc.sync.dma_start(
                    out=consts[:, TB : TB + MT],
                    in_=bg_d.rearrange("(c p) -> p c", p=P),
                )
                nc.sync.dma_start(
                    out=consts[:, TB + MT : TB + 2 * MT],
                    in_=bq_d.rearrange("(c p) -> p c", p=P),
                )
            decay_t = consts[:, 0:TB]

            xT_r = xT_d.rearrange("(c p) t -> p c t", p=P)
            l0_r = l0_buf.rearrange("(c p) t -> p c t", p=P)
            outT_r = outT_d.rearrange("(c p) t -> p c t", p=P)

            EQ = E // MQ  # 512
            ETQ = EQ // P  # 4 e-tiles per quarter
            wv_tiles = {}

            def load_wv(q):
                # Wv quarter; prefetched one phase ahead (tag has bufs=2)
                msl = slice(q * MQW, (q + 1) * MQW)
                t = wvp.tile([P, EC, MQW], F32R, tag="wv", name=f"wv{q}")
                nc.scalar.dma_start(
                    out=t, in_=wv_d[:, msl].rearrange("(c p) m -> p c m", p=P)
                )
                wv_tiles[("v", q)] = t

            def load_wo(eq):
                # Wo e-quarter; shares the wv tag / prefetch slot
                esl = slice(eq * EQ, (eq + 1) * EQ)
                t = wvp.tile([P, MT, EQ], F32R, tag="wv", name=f"wo{eq}")
                nc.scalar.dma_start(
                    out=t, in_=wo_d[:, esl].rearrange("(c p) e -> p c e", p=P)
                )
                wv_tiles[("o", eq)] = t

            def load_wga(q):
                # first half (m-tiles 0-1) of the Wg quarter; prefetched
                msl = slice(q * MQW, q * MQW + 2 * P)
                t = wgap.tile([P, EC, 2 * P], F32R, tag="wga", name=f"wga{q}")
                nc.scalar.dma_start(
                    out=t, in_=wg_d[:, msl].rearrange("(c p) m -> p c m", p=P)
                )
                wv_tiles[("gA", q)] = t

            # ---- Phases A0..A3: quarter q of m ----
            for q in range(MQ):
                msl = slice(q * MQW, (q + 1) * MQW)
                if q == 0:
                    # startup: first x block, then weights in order of need
                    xt0 = bigp.tile([P, EC, TB], F32R, tag="t16", name="xt0_0")
                    nc.sync.dma_start(out=xt0, in_=xT_r[:, :, 0:TB])
                    load_wv(0)
                    load_wga(0)
                wv = wv_tiles[("v", q)]
                wga = wv_tiles[("gA", q)]
                mem_prev = None
                for tb in range(NTB):
                    tsl = slice(tb * TB, (tb + 1) * TB)
                    if q == 0 and tb == 0:
                        xt = xt0
                    else:
                        xt = bigp.tile([P, EC, TB], F32R, tag="t16", name=f"xt{q}_{tb}")
                        nc.sync.dma_start(out=xt, in_=xT_r[:, :, tsl])
                    if tb == 0:
                        # rest of this phase's weights (after xt in queue order)
                        mslB = slice(q * MQW + 2 * P, (q + 1) * MQW)
                        wgb = wgbp.tile([P, EC, 2 * P], F32R, tag="wgb", name=f"wgb{q}")
                        nc.sync.dma_start(
                            out=wgb,
                            in_=wg_d[:, mslB].rearrange("(c p) m -> p c m", p=P),
                        )
                        wq = wqp.tile([P, EC, MQW], F32R, tag="wq", name=f"wq{q}")
                        nc.gpsimd.dma_start(
                            out=wq, in_=wq_d[:, msl].rearrange("(c p) m -> p c m", p=P)
                        )
                    if tb == 4:
                        # software prefetch of the next phase's first weights
                        if q + 1 < MQ:
                            load_wv(q + 1)
                            load_wga(q + 1)
                        else:
                            load_wo(0)
                    need_q = tb >= OTB0
                    mem_t = memp.tile([P, MT_Q, TB], F32, tag="mem", name=f"mem{q}_{tb}")
                    pvs = []
                    for mt in range(MT_Q):
                        pv = ps.tile(
                            [P, TB], F32, tag="pv", bufs=5, name=f"pv{q}_{tb}_{mt}"
                        )
                        for ec in range(EC):
                            nc.tensor.matmul(
                                pv, lhsT=wv[:, ec, wsl := slice(mt * P, (mt + 1) * P)],
                                rhs=xt[:, ec, :],
                                start=(ec == 0), stop=(ec == EC - 1),
                            )
                        pvs.append(pv)
                    wss = []
                    for mt in range(MT_Q):
                        mtg = q * MT_Q + mt  # global m tile
                        wsl = slice((mt % 2) * P, (mt % 2 + 1) * P)
                        wgt = wga if mt < 2 else wgb
                        ws = wsp.tile([P, 3, TB], F32R, tag="ws", name=f"ws{q}_{tb}_{mt}")
                        wss.append(ws)
                        gate, store = ws[:, 0, :], ws[:, 1, :]
                        pg = ps.tile([P, TB], F32, tag="pg", name=f"pg{q}_{tb}_{mt}")
                        for ec in range(EC):
                            nc.tensor.matmul(
                                pg, lhsT=wgt[:, ec, wsl], rhs=xt[:, ec, :],
                                start=(ec == 0), stop=(ec == EC - 1),
                            )
                        nc.scalar.activation(
                            gate, pg, mybir.ActivationFunctionType.Sigmoid,
                            bias=consts[:, TB + mtg : TB + mtg + 1] if has_bias else 0.0,
                        )
                        nc.vector.tensor_mul(store, pvs[mt], gate)
                        nc.vector.tensor_tensor_scan(
                            mem_t[:, mt, :], decay_t, store,
                            initial=0.0 if tb == 0 else mem_prev[:, mt, TB - 1 : TB],
                            op0=mybir.AluOpType.mult, op1=mybir.AluOpType.add,
                        )
                    if need_q:
                        osl = slice((tb - OTB0) * TB, (tb - OTB0 + 1) * TB)
                        for mt in range(MT_Q):
                            mtg = q * MT_Q + mt
                            wsl = slice(mt * P, (mt + 1) * P)
                            ws = wss[mt]
                            que, l0 = ws[:, 2, :], ws[:, 0, :]  # l0 reuses gate slot
                            pq = ps.tile(
                                [P, TB], F32, tag="pv", bufs=5, name=f"pq{q}_{tb}_{mt}"
                            )
                            for ec in range(EC):
                                nc.tensor.matmul(
                                    pq, lhsT=wq[:, ec, wsl], rhs=xt[:, ec, :],
                                    start=(ec == 0), stop=(ec == EC - 1),
                                )
                            nc.scalar.activation(
                                que, pq, mybir.ActivationFunctionType.Sigmoid,
                                bias=consts[:, TB + MT + mtg : TB + MT + mtg + 1]
                                if has_bias else 0.0,
                            )
                            nc.vector.tensor_mul(l0, mem_t[:, mt, :], que)
                            nc.gpsimd.dma_start(
                                out=l0_r[:, mtg : mtg + 1, osl],
                                in_=l0.unsqueeze(1),
                            )
                    mem_prev = mem_t

            # ---- Phases C: output projection, e-quarter PAIRS resident ----
            # token-block outer so each l0 block is read once per pair
            for cp in range(2):
                eqs = (2 * cp, 2 * cp + 1)
                # wo(2cp) prefetched earlier; its pair partner loads here
                load_wo(2 * cp + 1)
                wos = [wv_tiles[("o", eq)] for eq in eqs]
                for tb in range(OTB):
                    if cp == 0 and tb == 1:
                        load_wo(2)  # waits for a free slot (end of cp0)
                    tsl = slice(tb * TB, (tb + 1) * TB)
                    lt = bigp.tile([P, MT, TB], F32R, tag="t16", name=f"lt{cp}_{tb}")
                    nc.sync.dma_start(out=lt, in_=l0_r[:, :, tsl])
                    for j, eq in enumerate(eqs):
                        ot = memp.tile(
                            [P, ETQ, TB], F32, tag=("mem", "l0")[j],
                            name=f"ot{eq}_{tb}",
                        )
                        for et in range(ETQ):
                            po = ps.tile([P, TB], F32, tag="pg", name=f"po{eq}_{tb}_{et}")
                            for mc in range(MT):
                                nc.tensor.matmul(
                                    po, lhsT=wos[j][:, mc, et * P : (et + 1) * P],
                                    rhs=lt[:, mc, :],
                                    start=(mc == 0), stop=(mc == MT - 1),
                                )
                            nc.vector.tensor_copy(ot[:, et, :], po)
                        nc.gpsimd.dma_start(
                            out=outT_r[:, eq * ETQ : (eq + 1) * ETQ, tsl], in_=ot
                        )
    nc.compile()
    return nc


_cached = {}


def _get_module(has_bias):
    if has_bias not in _cached:
        _cached[has_bias] = build_module(has_bias)
    return _cached[has_bias]


def _prep_inputs(x, Wv, Wg, bg, Wq, bq, Wo, has_bias):
    """Shard + lay out host-side. Returns per-core input dicts."""
    x = np.asarray(x, dtype=np.float32)
    Wvs = (np.asarray(Wv, dtype=np.float32) * SCALE).astype(np.float32)
    Wos = (np.asarray(Wo, dtype=np.float32) * SCALE).astype(np.float32)
    Wg = np.ascontiguousarray(Wg, dtype=np.float32)
    Wq = np.ascontiguousarray(Wq, dtype=np.float32)
    in_maps = []
    for c in range(N_CORES):
        b, h = c // 2, c % 2
        xTc = np.zeros((E, T), dtype=np.float32)
        start = h * OUT_T - HALO
        src = np.ascontiguousarray(x[b, max(start, 0) : h * OUT_T + OUT_T].T)
        xTc[:, T - src.shape[1] :] = src
        m = {"xT": xTc, "Wvs": Wvs, "Wg": Wg, "Wq": Wq, "Wos": Wos}
        if has_bias:
            m["bg"] = np.ascontiguousarray(bg, dtype=np.float32)
            m["bq"] = np.ascontiguousarray(bq, dtype=np.float32)
        in_maps.append(m)
    return in_maps


def run(x, Wv, Wg, bg, Wq, bq, Wo, trace=False):
    bg = np.asarray(bg, dtype=np.float32)
    bq = np.asarray(bq, dtype=np.float32)
    has_bias = bool(np.any(bg)) or bool(np.any(bq))
    nc = _get_module(has_bias)
    in_maps = _prep_inputs(x, Wv, Wg, bg, Wq, bq, Wo, has_bias)
    res = run_bass_kernel_spmd(
        nc, in_maps, core_ids=list(range(N_CORES)), trace=trace
    )
    out = np.empty((B, S, E), dtype=np.float32)
    for c in range(N_CORES):
        b, h = c // 2, c % 2
        out[b, h * OUT_T : (h + 1) * OUT_T] = res.results[c]["outT"].T
    return out, res


def kernel(**inputs):
    out, _ = run(**inputs)
    return out


# revision 20
# speedup vs baseline: 1.1078x; 1.0039x over previous
"""Trainium2 Bass kernel for nn_Decay (gated decay-memory block).

  gate  = sigmoid(x @ Wg + bg)
  store = (x @ Wv) * gate * scale          scale = sqrt(1 - decay)
  mem   = decay-scan(store)                y_t = store_t + decay * y_{t-1}
  que   = sigmoid(x @ Wq + bq)
  out   = (mem * que * scale) @ Wo

Sharding (8 cores): core c handles batch b = c//2, token half h = c%2
(2048 output tokens each).  The decay scan needs history: each core
computes a 512-token halo before its token range (zero-padded for h=0,
so all cores run the identical program).  decay^512 ~ 4e-12 => exact to
fp32 precision.  No collectives.

Layout: everything on-chip lives as [feature (partitions), token (free)].
 - projections:  out[m_tile, t_blk] = sum_ec Wx[ec, m_tile].T @ xT[ec, t_blk]
   (weights in natural [E, M] layout; x transposed host-side)
 - decay scan: DVE tensor_tensor_scan along the free (token) axis
 - output proj consumes load0 [m, t] directly; result is outT [E, T],
   un-transposed host-side.
Matmuls run in float32r (TF32-like, full PE rate at N>=256).

Phases (weights resident in SBUF, activations streamed):
  A0..A3: m-quarter q of {Wv*scale, Wg, Wq} resident; computes
          pv, gate, store, mem(scan), que, load0 -> spill load0 (f32r)
  C0..C3: e-quarter of Wo*scale resident; outT[e, t] = sum_m Wo.T @ load0
Weight tiles use separate tags; the first-needed one (wv / wo) has
bufs=2 so the next phase's load overlaps the current phase's compute.
"""

import sys, types

sys.path.insert(0, "/opt/trn_rl_repo")

import numpy as np

import concourse.bass as bass
import concourse.tile as tile
from concourse import bacc, mybir
from concourse.bass_utils import run_bass_kernel_spmd

# Problem constants (hardcoded per harness contract)
B, S, E, M = 4, 4096, 2048, 2048
DECAY = 0.95
SCALE = float(np.sqrt(1.0 - DECAY))

N_CORES = 8
HALO = 256            # halo tokens ahead of each core's range (decay^256 ~ 2e-6)
OUT_T = S // 2        # output tokens per core
T = OUT_T + HALO      # computed tokens per core
TB = 256              # token block (matmul free dim)
NTB = T // TB         # 10
OTB = OUT_T // TB     # 8 output-token blocks
OTB0 = HALO // TB     # first t-block holding output tokens
P = 128
EC = E // P           # 16 contraction chunks
MT = M // P           # 16 m tiles
MQ = 4                # m-quarters
MT_Q = MT // MQ       # 4 m-tiles per quarter
MQW = MT_Q * P        # 512
F32 = mybir.dt.float32
F32R = mybir.dt.float32r


def build_module(has_bias):
    nc = bacc.Bacc()

    xT_d = nc.dram_tensor("xT", [E, T], F32R, kind="ExternalInput")
    wv_d = nc.dram_tensor("Wvs", [E, M], F32R, kind="ExternalInput")
    wg_d = nc.dram_tensor("Wg", [E, M], F32R, kind="ExternalInput")
    wq_d = nc.dram_tensor("Wq", [E, M], F32R, kind="ExternalInput")
    wo_d = nc.dram_tensor("Wos", [M, E], F32R, kind="ExternalInput")
    if has_bias:
        bg_d = nc.dram_tensor("bg", [M], F32, kind="ExternalInput")
        bq_d = nc.dram_tensor("bq", [M], F32, kind="ExternalInput")
    outT_d = nc.dram_tensor("outT", [E, OUT_T], F32, kind="ExternalOutput")
    l0_buf = nc.dram_tensor("l0_buf", [M, OUT_T], F32R)  # internal spill

    with tile.TileContext(nc) as tc:
        with (
            tc.tile_pool(name="wva", bufs=2) as wvp,   # Wv quarters + Wo quarters
            tc.tile_pool(name="wgap", bufs=2) as wgap,  # Wg half-quarters (prefetch)
            tc.tile_pool(name="wgbp", bufs=1) as wgbp,
            tc.tile_pool(name="wqp", bufs=1) as wqp,
            tc.tile_pool(name="big", bufs=2) as bigp,
            tc.tile_pool(name="ws", bufs=4) as wsp,
            tc.tile_pool(name="mems", bufs=2) as memp,
            tc.tile_pool(name="consts", bufs=1) as cp,
            tc.tile_pool(name="ps", bufs=2, space="PSUM") as ps,
        ):
            # consts: decay broadcast [:, :TB]; bg at [:, TB:TB+MT]; bq after
            consts = cp.tile([P, TB + 2 * MT], F32, tag="consts")
            nc.vector.memset(consts[:, 0:TB], DECAY)
            if has_bias:
                nc.sync.dma_start(
                    out=consts[:, TB : TB + MT],
                    in_=bg_d.rearrange("(c p) -> p c", p=P),
                )
                nc.sync.dma_start(
                    out=consts[:, TB + MT : TB + 2 * MT],
                    in_=bq_d.rearrange("(c p) -> p c", p=P),
                )
            decay_t = consts[:, 0:TB]

            xT_r = xT_d.rearrange("(c p) t -> p c t", p=P)
            l0_r = l0_buf.rearrange("(c p) t -> p c t", p=P)
            outT_r = outT_d.rearrange("(c p) t -> p c t", p=P)

            EQ = E // MQ  # 512
            ETQ = EQ // P  # 4 e-tiles per quarter
            wv_tiles = {}

            def load_wv(q):
                # Wv quarter; prefetched one phase ahead (tag has bufs=2)
                msl = slice(q * MQW, (q + 1) * MQW)
                t = wvp.tile([P, EC, MQW], F32R, tag="wv", name=f"wv{q}")
                nc.scalar.dma_start(
                    out=t, in_=wv_d[:, msl].rearrange("(c p) m -> p c m", p=P)
                )
                wv_tiles[("v", q)] = t

            def load_wo(eq):
                # Wo e-quarter; shares the wv tag / prefetch slot
                esl = slice(eq * EQ, (eq + 1) * EQ)
                t = wvp.tile([P, MT, EQ], F32R, tag="wv", name=f"wo{eq}")
                nc.scalar.dma_start(
                    out=t, in_=wo_d[:, esl].rearrange("(c p) e -> p c e", p=P)
                )
                wv_tiles[("o", eq)] = t

            def load_wga(q):
                # first half (m-tiles 0-1) of the Wg quarter; prefetched
                msl = slice(q * MQW, q * MQW + 2 * P)
                t = wgap.tile([P, EC, 2 * P], F32R, tag="wga", name=f"wga{q}")
                nc.scalar.dma_start(
                    out=t, in_=wg_d[:, msl].rearrange("(c p) m -> p c m", p=P)
                )
                wv_tiles[("gA", q)] = t

            # ---- Phases A0..A3: quarter q of m ----
            for q in range(MQ):
                msl = slice(q * MQW, (q + 1) * MQW)
                if q == 0:
                    # startup: first x block, then weights in order of need
                    xt0 = bigp.tile([P, EC, TB], F32R, tag="t16", name="xt0_0")
                    nc.sync.dma_start(out=xt0, in_=xT_r[:, :, 0:TB])
                    load_wv(0)
                    load_wga(0)
                wv = wv_tiles[("v", q)]
                wga = wv_tiles[("gA", q)]
                mem_prev = None
                for tb in range(NTB):
                    tsl = slice(tb * TB, (tb + 1) * TB)
                    if q == 0 and tb == 0:
                        xt = xt0
                    else:
                        xt = bigp.tile([P, EC, TB], F32R, tag="t16", name=f"xt{q}_{tb}")
                        nc.sync.dma_start(out=xt, in_=xT_r[:, :, tsl])
                    if tb == 0:
                        # rest of this phase's weights (after xt in queue order)
                        mslB = slice(q * MQW + 2 * P, (q + 1) * MQW)
                        wgb = wgbp.tile([P, EC, 2 * P], F32R, tag="wgb", name=f"wgb{q}")
                        nc.sync.dma_start(
                            out=wgb,
                            in_=wg_d[:, mslB].rearrange("(c p) m -> p c m", p=P),
                        )
                    if tb == 1:
                        # wq isn't needed until the first need_q block; delaying
                        # its load keeps boundary bandwidth for wgb/xt
                        wq = wqp.tile([P, EC, MQW], F32R, tag="wq", name=f"wq{q}")
                        nc.gpsimd.dma_start(
                            out=wq, in_=wq_d[:, msl].rearrange("(c p) m -> p c m", p=P)
                        )
                    if tb == 4:
                        # software prefetch of the next phase's first weights
                        if q + 1 < MQ:
                            load_wv(q + 1)
                            load_wga(q + 1)
                        else:
                            load_wo(0)
                    need_q = tb >= OTB0
                    mem_t = memp.tile([P, MT_Q, TB], F32, tag="mem", name=f"mem{q}_{tb}")
                    pvs = []
                    for mt in range(MT_Q):
                        pv = ps.tile(
                            [P, TB], F32, tag="pv", bufs=5, name=f"pv{q}_{tb}_{mt}"
                        )
                        for ec in range(EC):
                            nc.tensor.matmul(
                                pv, lhsT=wv[:, ec, wsl := slice(mt * P, (mt + 1) * P)],
                                rhs=xt[:, ec, :],
                                start=(ec == 0), stop=(ec == EC - 1),
                            )
                        pvs.append(pv)
                    wss = []
                    for mt in range(MT_Q):
                        mtg = q * MT_Q + mt  # global m tile
                        wsl = slice((mt % 2) * P, (mt % 2 + 1) * P)
                        wgt = wga if mt < 2 else wgb
                        ws = wsp.tile([P, 3, TB], F32R, tag="ws", name=f"ws{q}_{tb}_{mt}")
                        wss.append(ws)
                        gate, store = ws[:, 0, :], ws[:, 1, :]
                        pg = ps.tile([P, TB], F32, tag="pg", name=f"pg{q}_{tb}_{mt}")
                        for ec in range(EC):
                            nc.tensor.matmul(
                                pg, lhsT=wgt[:, ec, wsl], rhs=xt[:, ec, :],
                                start=(ec == 0), stop=(ec == EC - 1),
                            )
                        nc.scalar.activation(
                            gate, pg, mybir.ActivationFunctionType.Sigmoid,
                            bias=consts[:, TB + mtg : TB + mtg + 1] if has_bias else 0.0,
                        )
                        nc.vector.tensor_mul(store, pvs[mt], gate)
                        nc.vector.tensor_tensor_scan(
                            mem_t[:, mt, :], decay_t, store,
                            initial=0.0 if tb == 0 else mem_prev[:, mt, TB - 1 : TB],
                            op0=mybir.AluOpType.mult, op1=mybir.AluOpType.add,
                        )
                    if need_q:
                        osl = slice((tb - OTB0) * TB, (tb - OTB0 + 1) * TB)
                        for mt in range(MT_Q):
                            mtg = q * MT_Q + mt
                            wsl = slice(mt * P, (mt + 1) * P)
                            ws = wss[mt]
                            que, l0 = ws[:, 2, :], ws[:, 0, :]  # l0 reuses gate slot
                            pq = ps.tile(
                                [P, TB], F32, tag="pv", bufs=5, name=f"pq{q}_{tb}_{mt}"
                            )
                            for ec in range(EC):
                                nc.tensor.matmul(
                                    pq, lhsT=wq[:, ec, wsl], rhs=xt[:, ec, :],
                                    start=(ec == 0), stop=(ec == EC - 1),
                                )
                            nc.scalar.activation(
                                que, pq, mybir.ActivationFunctionType.Sigmoid,
                                bias=consts[:, TB + MT + mtg : TB + MT + mtg + 1]
                                if has_bias else 0.0,
                            )
                            nc.vector.tensor_mul(l0, mem_t[:, mt, :], que)
                            nc.gpsimd.dma_start(
                                out=l0_r[:, mtg : mtg + 1, osl],
                                in_=l0.unsqueeze(1),
                            )
                    mem_prev = mem_t

            # ---- Phases C: output projection, e-quarter PAIRS resident ----
            # token-block outer so each l0 block is read once per pair
            # wo1 borrows the (now free) wq slot so it loads during cp0's
            # first block instead of waiting for a wv-tag slot
            wo1 = wqp.tile([P, MT, EQ], F32R, tag="wq", name="wo1")
            nc.scalar.dma_start(
                out=wo1, in_=wo_d[:, EQ : 2 * EQ].rearrange("(c p) e -> p c e", p=P)
            )
            wv_tiles[("o", 1)] = wo1
            for cp in range(2):
                eqs = (2 * cp, 2 * cp + 1)
                wos = [wv_tiles[("o", eq)] for eq in eqs]
                for tb in range(OTB):
                    if cp == 0 and tb == 1:
                        load_wo(2)  # wv(A3)'s slot is free -> loads during cp0
                    if cp == 0 and tb == 4:
                        load_wo(3)  # second wv-tag slot frees at wo0... queued
                    tsl = slice(tb * TB, (tb + 1) * TB)
                    lt = bigp.tile([P, MT, TB], F32R, tag="t16", name=f"lt{cp}_{tb}")
                    nc.sync.dma_start(out=lt, in_=l0_r[:, :, tsl])
                    for j, eq in enumerate(eqs):
                        ot = memp.tile(
                            [P, ETQ, TB], F32, tag=("mem", "l0")[j],
                            name=f"ot{eq}_{tb}",
                        )
                        for et in range(ETQ):
                            po = ps.tile([P, TB], F32, tag="pg", name=f"po{eq}_{tb}_{et}")
                            for mc in range(MT):
                                nc.tensor.matmul(
                                    po, lhsT=wos[j][:, mc, et * P : (et + 1) * P],
                                    rhs=lt[:, mc, :],
                                    start=(mc == 0), stop=(mc == MT - 1),
                                )
                            nc.vector.tensor_copy(ot[:, et, :], po)
                        nc.gpsimd.dma_start(
                            out=outT_r[:, eq * ETQ : (eq + 1) * ETQ, tsl], in_=ot
                        )
    nc.compile()
    return nc


_cached = {}


def _get_module(has_bias):
    if has_bias not in _cached:
        _cached[has_bias] = build_module(has_bias)
    return _cached[has_bias]


def _prep_inputs(x, Wv, Wg, bg, Wq, bq, Wo, has_bias):
    """Shard + lay out host-side. Returns per-core input dicts."""
    x = np.asarray(x, dtype=np.float32)
    Wvs = (np.asarray(Wv, dtype=np.float32) * SCALE).astype(np.float32)
    Wos = (np.asarray(Wo, dtype=np.float32) * SCALE).astype(np.float32)
    Wg = np.ascontiguousarray(Wg, dtype=np.float32)
    Wq = np.ascontiguousarray(Wq, dtype=np.float32)
    in_maps = []
    for c in range(N_CORES):
        b, h = c // 2, c % 2
        xTc = np.zeros((E, T), dtype=np.float32)
        start = h * OUT_T - HALO
        src = np.ascontiguousarray(x[b, max(start, 0) : h * OUT_T + OUT_T].T)
        xTc[:, T - src.shape[1] :] = src
        m = {"xT": xTc, "Wvs": Wvs, "Wg": Wg, "Wq": Wq, "Wos": Wos}
        if has_bias:
            m["bg"] = np.ascontiguousarray(bg, dtype=np.float32)
            m["bq"] = np.ascontiguousarray(bq, dtype=np.float32)
        in_maps.append(m)
    return in_maps


def run(x, Wv, Wg, bg, Wq, bq, Wo, trace=False):
    bg = np.asarray(bg, dtype=np.float32)
    bq = np.asarray(bq, dtype=np.float32)
    has_bias = bool(np.any(bg)) or bool(np.any(bq))
    nc = _get_module(has_bias)
    in_maps = _prep_inputs(x, Wv, Wg, bg, Wq, bq, Wo, has_bias)
    res = run_bass_kernel_spmd(
        nc, in_maps, core_ids=list(range(N_CORES)), trace=trace
    )
    out = np.empty((B, S, E), dtype=np.float32)
    for c in range(N_CORES):
        b, h = c // 2, c % 2
        out[b, h * OUT_T : (h + 1) * OUT_T] = res.results[c]["outT"].T
    return out, res


def kernel(**inputs):
    out, _ = run(**inputs)
    return out
